# revision 9
# baseline (speedup 1.0000x reference)
"""DiffAttn TRN2 kernel (v2 — mode-batched PE + dual-engine exp).

out = (softmax(q1@k1.T/sqrt(4096)) - lam*softmax(q2@k2.T/sqrt(4096))) @ v
with q/k/v = x @ W{q,k,v}.T + b, q/k split into 32-dim halves.

Sharding: 8 cores = 2 batches x 4 Q-row-blocks (1024 rows each). Each core
recomputes K/V for its whole batch from x[b] (sequence order rolled so the
core's own Q block sits at columns 0:1024 of xT; softmax over keys is
permutation invariant so rolled K/V order does not change the result).

v2 design (vs v1 at ~101us):
  * PE groups are batched by tiling mode: consecutive matmuls in the same
    (row,col) tile mode hide their LDWEIGHTS in the background weight
    buffer; every mode switch costs a ~100ns array drain, so the loop
    does [scores ih0+ih1][AV x4][den quad][kv batch] = ~4 switches/chunk.
  * scores for the two q-column-blocks (ih) use DISJOINT PE row groups:
    ih0 at rows 0:64 (k1/k2 from kv_sb), ih1 at rows 64:128 (k1/k2
    replicated to partitions 64:128 of kvr_sb via SBUF->SBUF DMA), so the
    second pair's weight loads overlap the first pair's streams.
  * exp runs on BOTH Scalar (ACT spline, 1 op/gen) and Vector (4-op fp16
    minimax-cubic chain: u=a*x+b; sq=u*u; vv=c2*u+sq; p=(c1+vv)*u) —
    every KDG-th [128,1024] score tile goes to the DVE, the rest to ACT.
    Scores are tiny (|x| < 0.9), so the cubic is ~5e-3 pointwise and the
    softmax ratio cancels most of it.
  * probs are fp16 (better than bf16: exp in [0.4,2.6] is fp16-friendly).
  * den accumulates r/4096 = 1+delta via a 2^-12 ones-column matmul; the
    epilogue uses the division-free (1 - d + d^2)/4096 expansion.
"""

import math
import os

import numpy as np

import concourse.bass as bass
import concourse.bacc as bacc
import concourse.mybir as mybir
import concourse.tile as tile
from concourse.bass import ds, ts
from concourse.bass_utils import run_bass_kernel_spmd
from concourse.masks import make_identity

B, N, D, DK, DV, HALF = 2, 4096, 768, 64, 64, 32
NQ = N // 4  # q rows per core
NCH = D // 128  # 6 contraction chunks
NMC = N // 128  # 32 key chunks
NSL = N // 512  # 8 kv slices
F32 = mybir.dt.float32
BF16 = mybir.dt.bfloat16
FP16 = mybir.dt.float16

X_DT = BF16
X_NP = np.dtype("bfloat16")

Act = mybir.ActivationFunctionType
Alu = mybir.AluOpType

N_WARM = int(os.environ.get("KWARM", "30"))
# how many of the 64 [128,1024] gens run their exp on the DVE instead of ACT
N_DVE = int(os.environ.get("KNDVE", "13"))

# minimax cubic exp on [-0.95, 0.95] (max rel err 4.6e-3), factored so the
# chain uses only TS (4x fp16) and TT (2x fp16) DVE ops:
#   u = x + UC ; p = (u*u + WB) * (TA*u + TB)
UC = 0.7873273520676587
WB = 2.8774554474989027
TA = 0.1602151
TB = 0.15881384866723458


def _is_dve(mc: int, ih: int) -> bool:
    g = 2 * mc + ih
    return ((g + 1) * N_DVE) // 64 - (g * N_DVE) // 64 > 0


def _build() -> bass.Bass:
    nc = bacc.Bacc("TRN2", target_bir_lowering=False)

    # x eighths pre-transposed on host so each DMA is fully contiguous
    xT_d = nc.dram_tensor("xT", [NSL, 128, NCH, 512], X_DT, kind="ExternalInput")
    wkv_d = nc.dram_tensor("wkv", [128, NCH, 128], X_DT, kind="ExternalInput")
    wq_d = nc.dram_tensor("wq", [128, NCH, DK], X_DT, kind="ExternalInput")
    # packed per-partition constants: col0 = bkv, col1 = [bq*s; bq*s],
    # col2 = 1.0, col3 = -lam
    bc_d = nc.dram_tensor("bc", [128, 4], F32, kind="ExternalInput")
    # fp16 epilogue constants: col 0:64 = 1.0 rows, col 64:128 = -lam rows
    eb_d = nc.dram_tensor("eb", [128, 2 * DV], FP16, kind="ExternalInput")
    # stacked identity [I64; I64] for the epilogue combine matmul
    dbli_d = nc.dram_tensor("dbli", [128, DV], X_DT, kind="ExternalInput")
    # transposed output layout: host transposes back to [NQ, DV]
    out_d = nc.dram_tensor("out", [DV, NQ], F32, kind="ExternalOutput")

    with (
        tile.TileContext(nc) as tc,
        tc.tile_pool(name="const", bufs=1) as constp,
        tc.tile_pool(name="xp", bufs=1) as xp,
        tc.tile_pool(name="kvp", bufs=1) as kvp,
        tc.tile_pool(name="pp", bufs=4) as pp,
        tc.tile_pool(name="fin", bufs=1) as fin,
        tc.tile_pool(name="ps", bufs=2, space="PSUM") as ps,
        tc.tile_pool(name="us", bufs=1, space="PSUM") as us,
        tc.tile_pool(name="aux", bufs=1, space="PSUM") as aux,
    ):
        # ---- constants ----
        wkv_sb = constp.tile([128, NCH, 128], X_DT)
        wq_sb = constp.tile([128, NCH, DK], X_DT)
        bc_sb = constp.tile([128, 4], F32)
        eb_sb = constp.tile([128, 2 * DV], FP16)
        dbli_sb = constp.tile([128, DV], X_DT)
        bkv_sb = bc_sb[:, 0:1]
        bqq_sb = bc_sb[:, 1:2]
        ident_x = constp.tile([128, 128], X_DT)
        dencol = constp.tile([128, 1], X_DT)
        dummy = constp.tile([1, 1], F32)

        # x eighths 0/1 first (q projection needs both), then the small
        # weight DMAs, then the remaining x eighths
        x8 = [
            xp.tile([128, NCH, 512], X_DT, name=f"x_{e}", tag=f"x_{e}")
            for e in range(NSL)
        ]
        nc.sync.dma_start(out=x8[0], in_=xT_d[0])
        nc.sync.dma_start(out=x8[1], in_=xT_d[1])
        nc.sync.dma_start(out=wq_sb, in_=wq_d[:])
        nc.sync.dma_start(out=wkv_sb, in_=wkv_d[:])
        nc.sync.dma_start(out=bc_sb, in_=bc_d[:])
        nc.sync.dma_start(out=eb_sb, in_=eb_d[:])
        nc.sync.dma_start(out=dbli_sb, in_=dbli_d[:])
        for e in range(2, NSL):
            nc.sync.dma_start(out=x8[e], in_=xT_d[e])

        make_identity(nc, ident_x)
        # den column = 2^-12 so den accumulates r/4096 = 1+delta, |delta|<<1
        nc.vector.memset(dencol, 1.0 / 4096.0)
        nc.vector.memset(dummy, 1.0)
        nc.scalar.activation(out=dummy, in_=dummy, func=Act.Exp)

        kv_sb = kvp.tile([128, N], X_DT)
        # rows 64:96 = k1 copy, 96:128 = k2 copy (filled by SBUF->SBUF DMA)
        kvr_sb = kvp.tile([128, N], X_DT)
        vp_sb = kvp.tile([128, NMC, DV], BF16)
        # rows 0:32 q1b0, 32:64 q2b0, 64:96 q1b1, 96:128 q2b1
        qq_sb = kvp.tile([128, 512], X_DT)

        # ---- PE warm-up: ~3.2us of junk matmuls during the x-DMA wait trips
        # the HAM clock gate to 8/8 so the projection runs at 2.4 GHz ----
        warm = aux.tile([128, 128], F32, tag="aux", name="warm")
        for _ in range(N_WARM):
            nc.tensor.matmul(
                warm, lhsT=ident_x, rhs=ident_x, start=True, stop=True
            )

        # ---- kv projection for one 512-column slice: chunks [c0, c1) ----
        kv_state: dict = {}

        def kv_chunks(ms: int, c0: int, c1: int):
            if c0 == 0:
                kv_state[ms] = aux.tile(
                    [128, 512], F32, tag="aux", name=f"pkv{ms}"
                )
            for c in range(c0, c1):
                nc.tensor.matmul(
                    kv_state[ms],
                    lhsT=wkv_sb[:, c, :],
                    rhs=x8[ms][:, c, :],
                    start=(c == 0),
                    stop=(c == NCH - 1),
                    skip_group_check=True,
                )
            if c1 == NCH:
                nc.vector.tensor_scalar(
                    kv_sb[:, ts(ms, 512)],
                    kv_state.pop(ms),
                    bkv_sb,
                    None,
                    Alu.add,
                )
                # replicate k1/k2 to partitions 64:128 for the ih1 scores.
                # gpsimd software-DGE queue: empty, so this never queues
                # behind the big x-slice loads on the sync HWDGE queue
                nc.gpsimd.dma_start(
                    out=kvr_sb[64:128, ts(ms, 512)],
                    in_=kv_sb[0:64, ts(ms, 512)],
                )

        def vt_make(ms: int):
            vt = aux.tile([128, 4, DV], X_DT, tag="aux", name=f"vt{ms}")
            for j in range(4):
                nc.tensor.transpose(
                    out=vt[:, j, :],
                    in_=kv_sb[DV : 2 * DV, ts(4 * ms + j, 128)],
                    identity=ident_x[DV : 2 * DV, DV : 2 * DV],
                )
            nc.vector.tensor_copy(vp_sb[:, ds(4 * ms, 4), :], vt)

        # ---- prologue: kv slice 0, q projection (both blocks), vt(0) ----
        kv_chunks(0, 0, NCH)

        pq = us.tile([128, 512], F32, tag="u_0", name="pq")
        for c in range(NCH):
            nc.tensor.matmul(
                pq[0:DK, :],
                lhsT=wq_sb[:, c, :],
                rhs=x8[0][:, c, :],
                start=(c == 0),
                stop=(c == NCH - 1),
                tile_position=(0, 0),
                skip_group_check=True,
            )
            nc.tensor.matmul(
                pq[DK:128, :],
                lhsT=wq_sb[:, c, :],
                rhs=x8[1][:, c, :],
                start=(c == 0),
                stop=(c == NCH - 1),
                tile_position=(0, 64),
                skip_group_check=True,
            )
        nc.vector.tensor_scalar(qq_sb, pq, bqq_sb, None, Alu.add)
        vt_make(0)

        # ---- main loop ----
        uacc = [
            us.tile([128, 512], F32, tag=f"u_{ih}", name=f"u_{ih}")
            for ih in range(2)
        ]
        den = us.tile([128, 512], F32, tag="den", name="den")

        # schedules: ACT gens' exp completes during slot mc+1 -> AV at the
        # START of slot mc+2 sees it done; DVE chains finish during slot
        # mc+2 -> their AV runs at mc+3
        av_due: dict = {}
        den_due: dict = {}
        for mc in range(NMC):
            for ih in range(2):
                lag = 3 if _is_dve(mc, ih) else 2
                av_due.setdefault(mc + lag, []).append((mc, ih))
            dlag = 3 if (_is_dve(mc, 0) or _is_dve(mc, 1)) else 2
            den_due.setdefault(mc + dlag, []).append(mc)
        n_slots = max(max(av_due), max(den_due)) + 1

        # kv chunk batching: slice ms projected at slots 4(ms-1)+1 / +2,
        # v transposes at 4(ms-1)+3
        kv_slot: dict = {}
        vt_slot: dict = {}
        for ms in range(1, NSL):
            base = 4 * (ms - 1)
            kv_slot[base + 1] = (ms, 0, 3)
            kv_slot[base + 2] = (ms, 3, NCH)
            vt_slot[base + 3] = ms

        p_store: dict = {}
        dve_pend: dict = {}

        def scores_pair(mc: int, ih: int):
            """One 2-way row-tiled scores pair + its exp consumer.

            ih0 uses PE rows 0:64, ih1 rows 64:128 (disjoint row groups so
            consecutive pairs' weight loads overlap in-flight streams)."""
            s12 = ps.tile([128, 1024], F32, tag="sc", name=f"s{mc}_{ih}")
            r0 = 64 * ih
            lhs_src = kv_sb if ih == 0 else kvr_sb
            nc.tensor.matmul(
                s12[:, 0:512],
                lhsT=lhs_src[r0 : r0 + HALF, ts(mc, 128)],
                rhs=qq_sb[r0 : r0 + HALF, :],
                start=True,
                stop=True,
                tile_position=(r0, 0),
                skip_group_check=True,
            )
            nc.tensor.matmul(
                s12[:, 512:1024],
                lhsT=lhs_src[r0 + HALF : r0 + 64, ts(mc, 128)],
                rhs=qq_sb[r0 + HALF : r0 + 64, :],
                start=True,
                stop=True,
                tile_position=(r0 + HALF, 0),
                skip_group_check=True,
            )
            if _is_dve(mc, ih):
                u = pp.tile([128, 1024], FP16, tag="eu", name="eu", bufs=2)
                nc.vector.tensor_scalar(u, s12, UC, None, Alu.add)
                dve_pend[(mc, ih)] = u
            else:
                p12 = pp.tile(
                    [128, 1024], FP16, tag="p12", name="p12", bufs=8
                )
                nc.scalar.activation(out=p12, in_=s12, func=Act.Exp)
                p_store[(mc, ih)] = p12

        for slot in range(n_slots):
            mc = slot
            # a) scores ih0 early: its exp runs while the AV/den below
            # stream, so the PSUM buf recycles without stalling next slot
            if mc < NMC:
                scores_pair(mc, 0)
            # b) finish DVE chains started last slot (needed by AV at +2)
            for (m2, i2), u in [
                (k, v) for k, v in dve_pend.items() if k[0] == mc - 1
            ]:
                del dve_pend[(m2, i2)]
                sq = pp.tile([128, 1024], FP16, tag="esq", name="esq", bufs=2)
                w = pp.tile([128, 1024], FP16, tag="ew", name="ew", bufs=2)
                t = pp.tile([128, 1024], FP16, tag="et", name="et", bufs=2)
                p12 = pp.tile([128, 1024], FP16, tag="p12", name="p12", bufs=8)
                nc.vector.tensor_mul(sq, u, u)
                nc.vector.tensor_scalar(w, sq, WB, None, Alu.add)
                nc.vector.tensor_scalar(t, u, TA, TB, Alu.mult, Alu.add)
                nc.vector.tensor_mul(p12, w, t)
                p_store[(m2, i2)] = p12
            # c) AV pairs (col-tiled) for due gens
            for lm, jh in sorted(av_due.get(slot, [])):
                pt = p_store[(lm, jh)]
                u_ = uacc[jh]
                nc.tensor.matmul(
                    u_[0:DV, :],
                    lhsT=vp_sb[:, lm, :],
                    rhs=pt[:, 0:512],
                    start=(lm == 0),
                    stop=(lm == NMC - 1),
                    tile_position=(0, 0),
                    skip_group_check=True,
                )
                nc.tensor.matmul(
                    u_[DV:128, :],
                    lhsT=vp_sb[:, lm, :],
                    rhs=pt[:, 512:1024],
                    start=(lm == 0),
                    stop=(lm == NMC - 1),
                    tile_position=(0, 64),
                    skip_group_check=True,
                )
            # d) den quads
            for lm in den_due.get(slot, []):
                for jh in range(2):
                    pt = p_store[(lm, jh)]
                    for h in range(2):
                        r = 64 * jh + 32 * h
                        nc.tensor.matmul(
                            den[r : r + 1, :],
                            lhsT=dencol,
                            rhs=pt[:, ds(h * 512, 512)],
                            start=(lm == 0),
                            stop=(lm == NMC - 1),
                            tile_position=(0, r),
                            skip_group_check=True,
                        )
                for jh in range(2):
                    p_store.pop((lm, jh), None)
            # e) scores ih1 late: by now exp(mc-1, ih1) has drained its buf
            if mc < NMC:
                scores_pair(mc, 1)
            # f) kv projection batch / v transposes
            if slot in kv_slot:
                ms, c0, c1 = kv_slot[slot]
                kv_chunks(ms, c0, c1)
            if slot in vt_slot:
                vt_make(vt_slot[slot])

        # ---- epilogue ----
        # den = r/4096 = 1+delta with |delta| small, so
        # 1/r = (1 - delta + delta^2)/4096 to ~1e-4: three cheap DVE ops
        ud = fin.tile([128, 512], FP16, tag="ud", name="ud")
        ad = fin.tile([128, 512], FP16, tag="ad", name="ad")
        rec = fin.tile([128, 512], FP16, tag="rec", name="rec")
        nc.vector.tensor_scalar(ud, den, -1.0, None, Alu.add)
        nc.vector.scalar_tensor_tensor(ad, ud, -1.0, ud, Alu.add, Alu.mult)
        nc.vector.tensor_scalar(
            rec, ad, 1.0, 1.0 / 4096.0, Alu.add, Alu.mult
        )
        # PE broadcast: recb rows 0:64 = 1/r1, rows 64:128 = -lam/r2
        recb = ps.tile([128, 1024], F32, tag="sc", name="recb")
        for ih in range(2):
            r1, r2 = 64 * ih, 64 * ih + 32
            nc.tensor.matmul(
                recb[0:DV, ds(ih * 512, 512)],
                lhsT=eb_sb[r1 : r1 + 1, 0:DV],
                rhs=rec[r1 : r1 + 1, :],
                start=True,
                stop=True,
                tile_position=(r1, 0),
                skip_group_check=True,
            )
            nc.tensor.matmul(
                recb[DV:128, ds(ih * 512, 512)],
                lhsT=eb_sb[r2 : r2 + 1, DV : 2 * DV],
                rhs=rec[r2 : r2 + 1, :],
                start=True,
                stop=True,
                tile_position=(r2, 64),
                skip_group_check=True,
            )
        oo_ps = ps.tile([DV, NQ], F32, tag="sc", name="oo_ps")
        oo_sb = fin.tile([DV, NQ], F32, tag="oo", name="oo")
        for ih in range(2):
            recs = fin.tile([128, 512], F32, tag=f"recs{ih}", name=f"recs{ih}")
            nc.scalar.copy(recs, recb[:, ds(ih * 512, 512)])
            tm = fin.tile([128, 512], X_DT, tag=f"tm{ih}", name=f"tm{ih}")
            nc.vector.tensor_mul(tm, uacc[ih], recs)
            # oo[v, q] = tm[v, q] + tm[v+64, q] via stacked-identity matmul
            nc.tensor.matmul(
                oo_ps[:, ds(ih * 512, 512)],
                lhsT=dbli_sb,
                rhs=tm,
                start=True,
                stop=True,
                skip_group_check=True,
            )
            nc.scalar.copy(
                oo_sb[:, ds(ih * 512, 512)], oo_ps[:, ds(ih * 512, 512)]
            )
            nc.sync.dma_start(
                out=out_d[:, ds(ih * 512, 512)],
                in_=oo_sb[:, ds(ih * 512, 512)],
            )

    nc.finalize()
    return nc


_CACHE: dict = {}
LAST_RESULT = None


def _get_nc() -> bass.Bass:
    if "nc" not in _CACHE:
        _CACHE["nc"] = _build()
    return _CACHE["nc"]


def kernel(x, Wq, bq, Wk, bk, Wv, bv, lam) -> np.ndarray:
    global LAST_RESULT
    x = np.asarray(x, np.float32)
    Wq = np.asarray(Wq, np.float32)
    Wk = np.asarray(Wk, np.float32)
    Wv = np.asarray(Wv, np.float32)
    bq = np.asarray(bq, np.float32)
    bk = np.asarray(bk, np.float32)
    bv = np.asarray(bv, np.float32)
    lam_f = float(np.asarray(lam))

    s = 1.0 / math.sqrt(N)
    wq_h = np.ascontiguousarray(
        (Wq.T * s).astype(X_NP).reshape(NCH, 128, DK).transpose(1, 0, 2)
    )
    wkv_h = np.ascontiguousarray(
        np.concatenate([Wk.T, Wv.T], axis=1)
        .astype(X_NP)
        .reshape(NCH, 128, 128)
        .transpose(1, 0, 2)
    )
    bc_h = np.zeros((128, 4), np.float32)
    bc_h[:, 0] = np.concatenate([bk, bv])
    bc_h[:, 1] = np.concatenate([bq * s, bq * s])
    bc_h[:, 2] = 1.0
    bc_h[:, 3] = -lam_f
    eb_h = np.zeros((128, 2 * DV), np.float16)
    eb_h[:, 0:DV] = 1.0
    eb_h[:, DV : 2 * DV] = -lam_f
    dbli_h = np.concatenate([np.eye(DV), np.eye(DV)], axis=0).astype(X_NP)

    in_maps = []
    for core in range(8):
        b, blk = divmod(core, 4)
        xT = np.roll(x[b].T, -blk * NQ, axis=1).astype(X_NP)
        # [NSL, 128, NCH, 512]: each eighth fully contiguous for fast DMA
        xT = np.ascontiguousarray(
            xT.reshape(NCH, 128, NSL, 512).transpose(2, 1, 0, 3)
        )
        in_maps.append(
            dict(
                xT=xT,
                wkv=wkv_h,
                wq=wq_h,
                bc=bc_h,
                eb=eb_h,
                dbli=dbli_h,
            )
        )

    nc = _get_nc()
    res = run_bass_kernel_spmd(
        nc,
        in_maps,
        core_ids=list(range(8)),
        trace=os.environ.get("KTRACE", "0") == "1",
    )
    LAST_RESULT = res

    out = np.empty((B, N, DV), np.float32)
    for core in range(8):
        b, blk = divmod(core, 4)
        out[b, blk * NQ : (blk + 1) * NQ] = res.results[core]["out"].T
    return out


# revision 27
# speedup vs baseline: 1.1134x; 1.1134x over previous
"""DiffAttn TRN2 kernel (v2 — mode-batched PE + dual-engine exp).

out = (softmax(q1@k1.T/sqrt(4096)) - lam*softmax(q2@k2.T/sqrt(4096))) @ v
with q/k/v = x @ W{q,k,v}.T + b, q/k split into 32-dim halves.

Sharding: 8 cores = 2 batches x 4 Q-row-blocks (1024 rows each). Each core
recomputes K/V for its whole batch from x[b] (sequence order rolled so the
core's own Q block sits at columns 0:1024 of xT; softmax over keys is
permutation invariant so rolled K/V order does not change the result).

v2 design (vs v1 at ~101us):
  * PE groups are batched by tiling mode: consecutive matmuls in the same
    (row,col) tile mode hide their LDWEIGHTS in the background weight
    buffer; every mode switch costs a ~100ns array drain, so the loop
    does [scores ih0+ih1][AV x4][den quad][kv batch] = ~4 switches/chunk.
  * scores for the two q-column-blocks (ih) use DISJOINT PE row groups:
    ih0 at rows 0:64 (k1/k2 from kv_sb), ih1 at rows 64:128 (k1/k2
    replicated to partitions 64:128 of kvr_sb via SBUF->SBUF DMA), so the
    second pair's weight loads overlap the first pair's streams.
  * exp runs on BOTH Scalar (ACT spline, 1 op/gen) and Vector (4-op fp16
    minimax-cubic chain: u=a*x+b; sq=u*u; vv=c2*u+sq; p=(c1+vv)*u) —
    every KDG-th [128,1024] score tile goes to the DVE, the rest to ACT.
    Scores are tiny (|x| < 0.9), so the cubic is ~5e-3 pointwise and the
    softmax ratio cancels most of it.
  * probs are fp16 (better than bf16: exp in [0.4,2.6] is fp16-friendly).
  * den accumulates r/4096 = 1+delta via a 2^-12 ones-column matmul; the
    epilogue uses the division-free (1 - d + d^2)/4096 expansion.
"""

import math
import os

import numpy as np

import concourse.bass as bass
import concourse.bacc as bacc
import concourse.mybir as mybir
import concourse.tile as tile
from concourse.bass import ds, ts
from concourse.bass_utils import run_bass_kernel_spmd
from concourse.masks import make_identity

B, N, D, DK, DV, HALF = 2, 4096, 768, 64, 64, 32
NQ = N // 4  # q rows per core
NCH = D // 128  # 6 contraction chunks
NMC = N // 128  # 32 key chunks
NSL = N // 512  # 8 kv slices
F32 = mybir.dt.float32
BF16 = mybir.dt.bfloat16
FP16 = mybir.dt.float16

X_DT = BF16
X_NP = np.dtype("bfloat16")

Act = mybir.ActivationFunctionType
Alu = mybir.AluOpType

N_WARM = int(os.environ.get("KWARM", "30"))

# schedule configuration (overridable for offline timeline-sim sweeps)
CFG = dict(
    n_dve=int(os.environ.get("KNDVE", "13")),  # gens with exp on DVE (of 64)
    order=os.environ.get("KORD", "scores_last"),  # scores_last | stagger
    act_lag=int(os.environ.get("KACTLAG", "2")),  # AV lag for ACT gens
    dve_lag=int(os.environ.get("KDVELAG", "3")),  # AV lag for DVE gens
    kv_batch=os.environ.get("KKVB", "33"),  # "33": 3+3 chunks, "222": 2+2+2
    abl=frozenset(),  # sim-only ablations: decouple_exp | decouple_av
)

# minimax cubic exp on [-0.95, 0.95] (max rel err 4.6e-3), factored so the
# chain uses only TS (4x fp16) and TT (2x fp16) DVE ops:
#   u = x + UC ; p = (u*u + WB) * (TA*u + TB)
UC = 0.7873273520676587
WB = 2.8774554474989027
TA = 0.1602151
TB = 0.15881384866723458


def _is_dve(mc: int, ih: int) -> bool:
    g = 2 * mc + ih
    n = CFG["n_dve"]
    return ((g + 1) * n) // 64 - (g * n) // 64 > 0


def _build() -> bass.Bass:
    nc = bacc.Bacc("TRN2", target_bir_lowering=False)

    # x eighths pre-transposed on host so each DMA is fully contiguous
    xT_d = nc.dram_tensor("xT", [NSL, 128, NCH, 512], X_DT, kind="ExternalInput")
    wkv_d = nc.dram_tensor("wkv", [128, NCH, 128], X_DT, kind="ExternalInput")
    wq_d = nc.dram_tensor("wq", [128, NCH, DK], X_DT, kind="ExternalInput")
    # packed per-partition constants: col0 = bkv, col1 = [bq*s; bq*s],
    # col2 = 1.0, col3 = -lam
    bc_d = nc.dram_tensor("bc", [128, 4], F32, kind="ExternalInput")
    # fp16 epilogue constants: col 0:64 = 1.0 rows, col 64:128 = -lam rows
    eb_d = nc.dram_tensor("eb", [128, 2 * DV], FP16, kind="ExternalInput")
    # stacked identity [I64; I64] for the epilogue combine matmul
    dbli_d = nc.dram_tensor("dbli", [128, DV], X_DT, kind="ExternalInput")
    # transposed output layout: host transposes back to [NQ, DV]
    out_d = nc.dram_tensor("out", [DV, NQ], F32, kind="ExternalOutput")

    with (
        tile.TileContext(nc) as tc,
        tc.tile_pool(name="const", bufs=1) as constp,
        tc.tile_pool(name="xp", bufs=1) as xp,
        tc.tile_pool(name="kvp", bufs=1) as kvp,
        tc.tile_pool(name="pp", bufs=4) as pp,
        tc.tile_pool(name="fin", bufs=1) as fin,
        tc.tile_pool(name="ps", bufs=2, space="PSUM") as ps,
        tc.tile_pool(name="us", bufs=1, space="PSUM") as us,
        tc.tile_pool(name="aux", bufs=1, space="PSUM") as aux,
    ):
        # ---- constants ----
        wkv_sb = constp.tile([128, NCH, 128], X_DT)
        wq_sb = constp.tile([128, NCH, DK], X_DT)
        bc_sb = constp.tile([128, 4], F32)
        eb_sb = constp.tile([128, 2 * DV], FP16)
        dbli_sb = constp.tile([128, DV], X_DT)
        bkv_sb = bc_sb[:, 0:1]
        bqq_sb = bc_sb[:, 1:2]
        ident_x = constp.tile([128, 128], X_DT)
        dencol = constp.tile([128, 1], X_DT)
        dummy = constp.tile([1, 1], F32)

        # x eighths 0/1 first (q projection needs both), then the small
        # weight DMAs, then the remaining x eighths
        x8 = [
            xp.tile([128, NCH, 512], X_DT, name=f"x_{e}", tag=f"x_{e}")
            for e in range(NSL)
        ]
        nc.sync.dma_start(out=x8[0], in_=xT_d[0])
        nc.sync.dma_start(out=x8[1], in_=xT_d[1])
        nc.sync.dma_start(out=wq_sb, in_=wq_d[:])
        nc.sync.dma_start(out=wkv_sb, in_=wkv_d[:])
        nc.sync.dma_start(out=bc_sb, in_=bc_d[:])
        nc.sync.dma_start(out=eb_sb, in_=eb_d[:])
        nc.sync.dma_start(out=dbli_sb, in_=dbli_d[:])
        for e in range(2, NSL):
            nc.sync.dma_start(out=x8[e], in_=xT_d[e])

        make_identity(nc, ident_x)
        # den column = 2^-12 so den accumulates r/4096 = 1+delta, |delta|<<1
        nc.vector.memset(dencol, 1.0 / 4096.0)
        nc.vector.memset(dummy, 1.0)
        nc.scalar.activation(out=dummy, in_=dummy, func=Act.Exp)

        kv_sb = kvp.tile([128, N], X_DT)
        # rows 64:96 = k1 copy, 96:128 = k2 copy (filled by SBUF->SBUF DMA)
        kvr_sb = kvp.tile([128, N], X_DT)
        vp_sb = kvp.tile([128, NMC, DV], BF16)
        # rows 0:32 q1b0, 32:64 q2b0, 64:96 q1b1, 96:128 q2b1
        qq_sb = kvp.tile([128, 512], X_DT)

        # ---- PE warm-up: ~3.2us of junk matmuls during the x-DMA wait trips
        # the HAM clock gate to 8/8 so the projection runs at 2.4 GHz ----
        warm = aux.tile([128, 128], F32, tag="aux", name="warm")
        for _ in range(N_WARM):
            nc.tensor.matmul(
                warm, lhsT=ident_x, rhs=ident_x, start=True, stop=True
            )

        # ---- kv projection for one 512-column slice: chunks [c0, c1) ----
        kv_state: dict = {}

        def kv_chunks(ms: int, c0: int, c1: int):
            if c0 == 0:
                kv_state[ms] = aux.tile(
                    [128, 512], F32, tag="aux", name=f"pkv{ms}"
                )
            for c in range(c0, c1):
                nc.tensor.matmul(
                    kv_state[ms],
                    lhsT=wkv_sb[:, c, :],
                    rhs=x8[ms][:, c, :],
                    start=(c == 0),
                    stop=(c == NCH - 1),
                    skip_group_check=True,
                )
            if c1 == NCH:
                nc.vector.tensor_scalar(
                    kv_sb[:, ts(ms, 512)],
                    kv_state.pop(ms),
                    bkv_sb,
                    None,
                    Alu.add,
                )
                # replicate k1/k2 to partitions 64:128 for the ih1 scores.
                # gpsimd software-DGE queue: empty, so this never queues
                # behind the big x-slice loads on the sync HWDGE queue
                nc.gpsimd.dma_start(
                    out=kvr_sb[64:128, ts(ms, 512)],
                    in_=kv_sb[0:64, ts(ms, 512)],
                )

        def vt_make(ms: int):
            vt = aux.tile([128, 4, DV], X_DT, tag="aux", name=f"vt{ms}")
            for j in range(4):
                nc.tensor.transpose(
                    out=vt[:, j, :],
                    in_=kv_sb[DV : 2 * DV, ts(4 * ms + j, 128)],
                    identity=ident_x[DV : 2 * DV, DV : 2 * DV],
                )
            nc.vector.tensor_copy(vp_sb[:, ds(4 * ms, 4), :], vt)

        # ---- prologue: kv slice 0, q projection (both blocks), vt(0) ----
        kv_chunks(0, 0, NCH)

        pq = us.tile([128, 512], F32, tag="u_0", name="pq")
        for c in range(NCH):
            nc.tensor.matmul(
                pq[0:DK, :],
                lhsT=wq_sb[:, c, :],
                rhs=x8[0][:, c, :],
                start=(c == 0),
                stop=(c == NCH - 1),
                tile_position=(0, 0),
                skip_group_check=True,
            )
            nc.tensor.matmul(
                pq[DK:128, :],
                lhsT=wq_sb[:, c, :],
                rhs=x8[1][:, c, :],
                start=(c == 0),
                stop=(c == NCH - 1),
                tile_position=(0, 64),
                skip_group_check=True,
            )
        nc.vector.tensor_scalar(qq_sb, pq, bqq_sb, None, Alu.add)

        # ---- main loop ----
        uacc = [
            us.tile([128, 512], F32, tag=f"u_{ih}", name=f"u_{ih}")
            for ih in range(2)
        ]
        den = us.tile([128, 512], F32, tag="den", name="den")

        # schedules: ACT gens' exp completes during slot mc+1 -> AV at the
        # START of slot mc+2 sees it done; DVE chains finish during slot
        # mc+2 -> their AV runs at mc+3
        alag, dlag_ = CFG["act_lag"], CFG["dve_lag"]
        av_due: dict = {}
        den_due: dict = {}
        for mc in range(NMC):
            for ih in range(2):
                lag = dlag_ if _is_dve(mc, ih) else alag
                av_due.setdefault(mc + lag, []).append((mc, ih))
            dl = dlag_ if (_is_dve(mc, 0) or _is_dve(mc, 1)) else alag
            den_due.setdefault(mc + dl, []).append(mc)
        n_slots = max(max(av_due), max(den_due)) + 1

        # kv chunk batching: slice ms projected early in its 4-slot window,
        # v transposes at 4(ms-1)+3
        kv_slot: dict = {}
        vt_slot: dict = {}
        for ms in range(1, NSL):
            base = 4 * (ms - 1)
            if CFG["kv_batch"] == "33":
                kv_slot[base + 1] = [(ms, 0, 3)]
                kv_slot[base + 2] = [(ms, 3, NCH)]
            else:
                kv_slot[base + 0] = [(ms, 0, 2)]
                kv_slot[base + 1] = [(ms, 2, 4)]
                kv_slot[base + 2] = [(ms, 4, NCH)]
            vt_slot[base + 3] = ms

        p_store: dict = {}
        dve_pend: dict = {}
        ps_dummy = kvp.tile([128, 1024], F32, name="ps_dummy")
        pp_dummy = kvp.tile([128, 1024], FP16, name="pp_dummy")
        if CFG["abl"]:
            nc.vector.memset(ps_dummy, 0.001)
            nc.vector.memset(pp_dummy, 0.001)

        def scores_pair(mc: int, ih: int):
            """One 2-way row-tiled scores pair + its exp consumer.

            ih0 uses PE rows 0:64, ih1 rows 64:128 (disjoint row groups so
            consecutive pairs' weight loads overlap in-flight streams)."""
            if "no_scores" in CFG["abl"]:
                if "no_exp" not in CFG["abl"]:
                    pass
                p_store[(mc, ih)] = pp_dummy
                return
            s12 = ps.tile([128, 1024], F32, tag="sc", name=f"s{mc}_{ih}")
            r0 = 64 * ih
            lhs_src = kv_sb if ih == 0 else kvr_sb
            nc.tensor.matmul(
                s12[:, 0:512],
                lhsT=lhs_src[r0 : r0 + HALF, ts(mc, 128)],
                rhs=qq_sb[r0 : r0 + HALF, :],
                start=True,
                stop=True,
                tile_position=(r0, 0),
                skip_group_check=True,
            )
            nc.tensor.matmul(
                s12[:, 512:1024],
                lhsT=lhs_src[r0 + HALF : r0 + 64, ts(mc, 128)],
                rhs=qq_sb[r0 + HALF : r0 + 64, :],
                start=True,
                stop=True,
                tile_position=(r0 + HALF, 0),
                skip_group_check=True,
            )
            if "decouple_exp" in CFG["abl"]:
                s12 = ps_dummy  # exp reads a constant: breaks scores->exp dep
            if "no_exp" in CFG["abl"]:
                p_store[(mc, ih)] = pp_dummy
                return
            if _is_dve(mc, ih):
                u = pp.tile([128, 1024], FP16, tag="eu", name="eu", bufs=2)
                nc.vector.tensor_scalar(u, s12, UC, None, Alu.add)
                dve_pend[(mc, ih)] = u
            else:
                p12 = pp.tile(
                    [128, 1024], FP16, tag="p12", name="p12", bufs=8
                )
                nc.scalar.activation(out=p12, in_=s12, func=Act.Exp)
                p_store[(mc, ih)] = p12
            if "decouple_av" in CFG["abl"]:
                p_store[(mc, ih)] = pp_dummy  # AV/den read a constant

        def dve_finish(mc: int):
            for (m2, i2), u in [
                (k, v) for k, v in dve_pend.items() if k[0] == mc - 1
            ]:
                del dve_pend[(m2, i2)]
                sq = pp.tile([128, 1024], FP16, tag="esq", name="esq", bufs=2)
                w = pp.tile([128, 1024], FP16, tag="ew", name="ew", bufs=2)
                t = pp.tile([128, 1024], FP16, tag="et", name="et", bufs=2)
                p12 = pp.tile([128, 1024], FP16, tag="p12", name="p12", bufs=8)
                nc.vector.tensor_mul(sq, u, u)
                nc.vector.tensor_scalar(w, sq, WB, None, Alu.add)
                nc.vector.tensor_scalar(t, u, TA, TB, Alu.mult, Alu.add)
                nc.vector.tensor_mul(p12, w, t)
                p_store[(m2, i2)] = p12

        def av_run(slot: int):
            if "no_avden" in CFG["abl"]:
                return
            for lm, jh in sorted(av_due.get(slot, [])):
                pt = p_store[(lm, jh)]
                u_ = uacc[jh]
                nc.tensor.matmul(
                    u_[0:DV, :],
                    lhsT=vp_sb[:, lm, :],
                    rhs=pt[:, 0:512],
                    start=(lm == 0),
                    stop=(lm == NMC - 1),
                    tile_position=(0, 0),
                    skip_group_check=True,
                )
                nc.tensor.matmul(
                    u_[DV:128, :],
                    lhsT=vp_sb[:, lm, :],
                    rhs=pt[:, 512:1024],
                    start=(lm == 0),
                    stop=(lm == NMC - 1),
                    tile_position=(0, 64),
                    skip_group_check=True,
                )

        def den_run(slot: int):
            if "no_avden" in CFG["abl"]:
                for lm in den_due.get(slot, []):
                    for jh in range(2):
                        p_store.pop((lm, jh), None)
                return
            for lm in den_due.get(slot, []):
                for jh in range(2):
                    pt = p_store[(lm, jh)]
                    for h in range(2):
                        r = 64 * jh + 32 * h
                        nc.tensor.matmul(
                            den[r : r + 1, :],
                            lhsT=dencol,
                            rhs=pt[:, ds(h * 512, 512)],
                            start=(lm == 0),
                            stop=(lm == NMC - 1),
                            tile_position=(0, r),
                            skip_group_check=True,
                        )
                for jh in range(2):
                    p_store.pop((lm, jh), None)

        def kv_run(slot: int):
            for ms, c0, c1 in kv_slot.get(slot, []):
                kv_chunks(ms, c0, c1)
            if slot in vt_slot:
                vt_make(vt_slot[slot])

        for slot in range(n_slots):
            mc = slot
            if CFG["order"] == "stagger":
                if mc < NMC:
                    scores_pair(mc, 0)
                dve_finish(mc)
                av_run(slot)
                den_run(slot)
                if mc < NMC:
                    scores_pair(mc, 1)
                kv_run(slot)
            else:  # scores_last
                dve_finish(mc)
                av_run(slot)
                den_run(slot)
                kv_run(slot)
                if mc < NMC:
                    scores_pair(mc, 0)
                    scores_pair(mc, 1)
            if slot == 0:
                # v transposes for slice 0, deferred past scores(0) so the
                # first exp starts as early as possible
                vt_make(0)

        # ---- epilogue ----
        # den = r/4096 = 1+delta with |delta| small, so
        # 1/r = (1 - delta + delta^2)/4096 to ~1e-4: three cheap DVE ops
        ud = fin.tile([128, 512], FP16, tag="ud", name="ud")
        ad = fin.tile([128, 512], FP16, tag="ad", name="ad")
        rec = fin.tile([128, 512], FP16, tag="rec", name="rec")
        nc.vector.tensor_scalar(ud, den, -1.0, None, Alu.add)
        nc.vector.scalar_tensor_tensor(ad, ud, -1.0, ud, Alu.add, Alu.mult)
        nc.vector.tensor_scalar(
            rec, ad, 1.0, 1.0 / 4096.0, Alu.add, Alu.mult
        )
        # PE broadcast: recb rows 0:64 = 1/r1, rows 64:128 = -lam/r2
        recb = ps.tile([128, 1024], F32, tag="sc", name="recb")
        for ih in range(2):
            r1, r2 = 64 * ih, 64 * ih + 32
            nc.tensor.matmul(
                recb[0:DV, ds(ih * 512, 512)],
                lhsT=eb_sb[r1 : r1 + 1, 0:DV],
                rhs=rec[r1 : r1 + 1, :],
                start=True,
                stop=True,
                tile_position=(r1, 0),
                skip_group_check=True,
            )
            nc.tensor.matmul(
                recb[DV:128, ds(ih * 512, 512)],
                lhsT=eb_sb[r2 : r2 + 1, DV : 2 * DV],
                rhs=rec[r2 : r2 + 1, :],
                start=True,
                stop=True,
                tile_position=(r2, 64),
                skip_group_check=True,
            )
        oo_ps = ps.tile([DV, NQ], F32, tag="sc", name="oo_ps")
        oo_sb = fin.tile([DV, NQ], F32, tag="oo", name="oo")
        for ih in range(2):
            recs = fin.tile([128, 512], F32, tag=f"recs{ih}", name=f"recs{ih}")
            nc.scalar.copy(recs, recb[:, ds(ih * 512, 512)])
            tm = fin.tile([128, 512], X_DT, tag=f"tm{ih}", name=f"tm{ih}")
            nc.vector.tensor_mul(tm, uacc[ih], recs)
            # oo[v, q] = tm[v, q] + tm[v+64, q] via stacked-identity matmul
            nc.tensor.matmul(
                oo_ps[:, ds(ih * 512, 512)],
                lhsT=dbli_sb,
                rhs=tm,
                start=True,
                stop=True,
                skip_group_check=True,
            )
            nc.scalar.copy(
                oo_sb[:, ds(ih * 512, 512)], oo_ps[:, ds(ih * 512, 512)]
            )
            nc.sync.dma_start(
                out=out_d[:, ds(ih * 512, 512)],
                in_=oo_sb[:, ds(ih * 512, 512)],
            )

    nc.finalize()
    return nc


_CACHE: dict = {}
LAST_RESULT = None


def _get_nc() -> bass.Bass:
    if "nc" not in _CACHE:
        _CACHE["nc"] = _build()
    return _CACHE["nc"]


def kernel(x, Wq, bq, Wk, bk, Wv, bv, lam) -> np.ndarray:
    global LAST_RESULT
    x = np.asarray(x, np.float32)
    Wq = np.asarray(Wq, np.float32)
    Wk = np.asarray(Wk, np.float32)
    Wv = np.asarray(Wv, np.float32)
    bq = np.asarray(bq, np.float32)
    bk = np.asarray(bk, np.float32)
    bv = np.asarray(bv, np.float32)
    lam_f = float(np.asarray(lam))

    s = 1.0 / math.sqrt(N)
    wq_h = np.ascontiguousarray(
        (Wq.T * s).astype(X_NP).reshape(NCH, 128, DK).transpose(1, 0, 2)
    )
    wkv_h = np.ascontiguousarray(
        np.concatenate([Wk.T, Wv.T], axis=1)
        .astype(X_NP)
        .reshape(NCH, 128, 128)
        .transpose(1, 0, 2)
    )
    bc_h = np.zeros((128, 4), np.float32)
    bc_h[:, 0] = np.concatenate([bk, bv])
    bc_h[:, 1] = np.concatenate([bq * s, bq * s])
    bc_h[:, 2] = 1.0
    bc_h[:, 3] = -lam_f
    eb_h = np.zeros((128, 2 * DV), np.float16)
    eb_h[:, 0:DV] = 1.0
    eb_h[:, DV : 2 * DV] = -lam_f
    dbli_h = np.concatenate([np.eye(DV), np.eye(DV)], axis=0).astype(X_NP)

    in_maps = []
    for core in range(8):
        b, blk = divmod(core, 4)
        xT = np.roll(x[b].T, -blk * NQ, axis=1).astype(X_NP)
        # [NSL, 128, NCH, 512]: each eighth fully contiguous for fast DMA
        xT = np.ascontiguousarray(
            xT.reshape(NCH, 128, NSL, 512).transpose(2, 1, 0, 3)
        )
        in_maps.append(
            dict(
                xT=xT,
                wkv=wkv_h,
                wq=wq_h,
                bc=bc_h,
                eb=eb_h,
                dbli=dbli_h,
            )
        )

    nc = _get_nc()
    res = run_bass_kernel_spmd(
        nc,
        in_maps,
        core_ids=list(range(8)),
        trace=os.environ.get("KTRACE", "0") == "1",
    )
    LAST_RESULT = res

    out = np.empty((B, N, DV), np.float32)
    for core in range(8):
        b, blk = divmod(core, 4)
        out[b, blk * NQ : (blk + 1) * NQ] = res.results[core]["out"].T
    return out
